# revision 1
# baseline (speedup 1.0000x reference)
"""GCN (2x GCNConv + mean-pool + linear) on 8 Trainium2 NeuronCores.

Sharding: nodes (and their incoming edges) are sharded by contiguous
dst-node ranges across the 8 cores; weights are replicated.  Each layer's
aggregation runs as: dma_gather of 256B bf16 feature rows (edge messages)
-> per-chunk one-hot built on DVE (iota == dst_local) -> PE matmul
scatter-add accumulated in PSUM -> dense transform + relu + epilogue.

Normalization algebra: with deg including the self loop,
  gcn(x)_i = relu( dinsq_i * (W.T @ sum_{e->i} dinsq_src * x_src) + b )
where the self loop is just one more edge.  relu(s*x) = s*relu(x) for s>0
lets the dst-side dinsq commute out of the relu; the bias enters the PSUM
accumulation as a rank-1 matmul b (x) sqrtdeg.  The mean-pool + dst scale
of layer 2 fold into a single valued indicator matmul.
"""

import sys
from contextlib import ExitStack

for _p in ("/opt/trn_rl_repo",):
    if _p not in sys.path:
        sys.path.insert(0, _p)

import numpy as np
import ml_dtypes

import concourse.bass as bass
import concourse.mybir as mybir
import concourse.tile as tile
from concourse import bacc
from concourse.bass_utils import run_bass_kernel_spmd
from concourse.library_config import mlp

BF16 = mybir.dt.float16
F32 = mybir.dt.float32
I16 = mybir.dt.int16
BF16_NP = np.float16


class Cfg:
    def __init__(self, N=100000, E=1600000, G=100, DIN=3, H=128, OUT=10,
                 NCORES=8, WT=8, SC_SIZE=25000):
        self.N, self.E, self.G = N, E, G
        self.DIN, self.H, self.OUT = DIN, H, OUT
        self.NCORES = NCORES
        assert N % NCORES == 0
        self.NPC = N // NCORES                      # nodes per core
        self.NT = (self.NPC + 127) // 128           # dst tiles per core
        self.LAST_VALID = self.NPC - (self.NT - 1) * 128
        self.WT = WT                                # tiles per wave
        self.NW = (self.NT + WT - 1) // WT
        assert SC_SIZE <= 32768
        self.SC_SIZE = SC_SIZE                      # src chunk rows (int16 idx)
        self.NSC = (N + SC_SIZE - 1) // SC_SIZE


FULL = Cfg()


# --------------------------------------------------------------------------
# host preprocessing
# --------------------------------------------------------------------------

def preprocess(cfg, x, edge_index, batch):
    N, G, NC = cfg.N, cfg.G, cfg.NCORES
    NPC, NT, NSC, WT = cfg.NPC, cfg.NT, cfg.NSC, cfg.WT
    src = np.asarray(edge_index[0], dtype=np.int64)
    dst = np.asarray(edge_index[1], dtype=np.int64)
    batch = np.asarray(batch, dtype=np.int64)
    x = np.asarray(x, dtype=np.float32)

    deg = (np.bincount(dst, minlength=N) + 1.0).astype(np.float32)
    dinsq = (1.0 / np.sqrt(deg)).astype(np.float32)
    invdeg = (dinsq * dinsq).astype(np.float32)
    sqrtdeg = np.sqrt(deg).astype(np.float32)
    cnt = np.bincount(batch, minlength=G).astype(np.float32)
    invcnt = (1.0 / np.maximum(cnt, 1.0)).astype(np.float32)

    # gather table for layer 1: bf16 x rows scaled by dinsq, padded to 128
    x_pad = np.zeros((N, 128), dtype=BF16_NP)
    x_pad[:, :cfg.DIN] = (x * dinsq[:, None]).astype(BF16_NP)

    # self loops are handled by a per-tile identity matmul on the tile's own
    # table rows, not as gathered edges (avoids their padding clustering)
    src_all = src
    dst_all = dst

    core = dst_all // NPC
    dst_local = dst_all - core * NPC
    tl = dst_local >> 7
    sc = src_all // cfg.SC_SIZE
    key = (core * NT + tl) * NSC + sc
    order = np.argsort(key, kind="stable")
    key_s = key[order]
    src_s = src_all[order]
    sc_s = sc[order]

    counts = np.bincount(key_s, minlength=NC * NT * NSC).reshape(NC, NT, NSC)
    # per-(tile, srcchunk) slot count, padded to 32 and uniform across cores
    GRAIN = 32
    P = ((counts.max(axis=0) + GRAIN - 1) // GRAIN * GRAIN).astype(np.int64)

    waves = [list(range(w * WT, min((w + 1) * WT, NT))) for w in range(cfg.NW)]
    # group = (wave, srcchunk): tiles' slot ranges concatenated, chunked by 128
    slot_base = np.zeros((NT, NSC), dtype=np.int64)   # global slot index
    gmeta = []     # per wave: per s: (idx_col0, nidx, msgcol0, nch)
    wmms = []      # per wave: ordered list of (mcol, j_in_wave, gcol, tile)
    pos = 0        # global chunk counter
    SENT = 16384.0
    for w, wtiles in enumerate(waves):
        wmeta = []
        wave_chunk0 = pos
        mms = []
        for s in range(NSC):
            c0 = pos
            off = 0     # slot offset within group
            spans = []  # (t, slot_lo, slot_hi) within group
            for t in wtiles:
                slot_base[t, s] = c0 * 128 + off
                if P[t, s]:
                    spans.append((t, off, off + int(P[t, s])))
                off += int(P[t, s])
            nch = (off + 127) // 128
            for k in range(nch):
                lo, hi = k * 128, (k + 1) * 128
                sp = [t for t, a, b in spans if a < hi and b > lo]
                if not sp:
                    continue
                j0, j1 = sp[0] - wtiles[0], sp[-1] - wtiles[0]
                jj = j0
                while jj <= j1:          # split windows at psum-bank groups
                    je = min(j1, (jj // 4) * 4 + 3)
                    mms.append((c0 - wave_chunk0 + k, jj, je - jj + 1, c0 + k))
                    jj = je + 1
            pos += nch
            wmeta.append((c0 * 8, nch * 128, c0 - wave_chunk0, nch))
        gmeta.append(wmeta)
        wmms.append(mms)
    TOTCH = pos
    CW = max(sum(gmeta[w][s][3] for s in range(NSC))
             for w in range(cfg.NW))

    # scatter edge data into padded per-core arrays
    idx_all = np.zeros((NC, TOTCH * 128), dtype=np.int16)
    dstl_all = np.full((NC, TOTCH * 128), SENT, dtype=np.float32)
    bstart = np.zeros(NC * NT * NSC, dtype=np.int64)
    cflat = counts.reshape(-1)
    bstart[1:] = np.cumsum(cflat)[:-1]
    rank = np.arange(len(key_s)) - bstart[key_s]
    ccore = key_s // (NT * NSC)
    rem = key_s % (NT * NSC)
    dest = slot_base.reshape(-1)[rem] + rank
    idx_all[ccore, dest] = (src_s - sc_s * cfg.SC_SIZE).astype(np.int16)
    # dst index relative to the wave's first tile
    wavebase = (tl[order] // WT) * WT * 128
    dstl_all[ccore, dest] = (dst_local[order] - wavebase).astype(np.float32)

    idx_wrap = np.ascontiguousarray(
        np.tile(idx_all.reshape(NC, TOTCH * 8, 16).transpose(0, 2, 1), (1, 8, 1)))
    dstl_wrap = np.ascontiguousarray(
        dstl_all.reshape(NC, TOTCH, 128).transpose(0, 2, 1))

    # per-core per-node columns/rows (padded to NT*128)
    NPAD = NT * 128
    def core_nodes(c):
        idx = np.arange(NPAD) + c * NPC
        valid = np.arange(NPAD) < NPC
        idx = np.where(valid, idx, 0)
        return idx, valid

    invdeg_col = np.zeros((NC, 128, NT), dtype=np.float32)
    sqrtdeg_row = np.ones((NC, 1, NPAD), dtype=np.float32)
    poolw = np.zeros((NC, NT, 128, G), dtype=np.float32)
    for c in range(NC):
        idx, valid = core_nodes(c)
        iv = np.where(valid, invdeg[idx], 1.0).astype(np.float32)
        invdeg_col[c] = iv.reshape(NT, 128).T
        sqrtdeg_row[c, 0] = np.where(valid, sqrtdeg[idx], 1.0)
        wv = np.where(valid, dinsq[idx] * invcnt[batch[idx]], 0.0)
        g_of = batch[idx]
        t_of = np.arange(NPAD) >> 7
        p_of = np.arange(NPAD) & 127
        m = valid
        poolw[c, t_of[m], p_of[m], g_of[m]] = wv[m]

    # wave-relative iota: iota[p, j*128 + n] = j*128 + n
    iota = np.tile(np.arange(WT * 128, dtype=np.float32), (128, 1)).astype(BF16_NP)
    ident = np.eye(128, dtype=np.float32)

    return dict(
        x_pad=x_pad, idx_wrap=idx_wrap, dstl_wrap=dstl_wrap,
        invdeg_col=invdeg_col, sqrtdeg_row=sqrtdeg_row, poolw=poolw,
        iota=iota, ident=ident, waves=waves, gmeta=gmeta, wmms=wmms,
        TOTCH=TOTCH, CW=CW, deg=deg,
    )


# --------------------------------------------------------------------------
# kernel builder (one GCN layer; layer 2 also does pooling + final linear)
# --------------------------------------------------------------------------

def build_layer(cfg, meta, layer, has_bias):
    N = cfg.N
    NT, NSC, WT, NPC = cfg.NT, cfg.NSC, cfg.WT, cfg.NPC
    TOTCH, CW = meta["TOTCH"], meta["CW"]
    waves, gmeta, wmms = meta["waves"], meta["gmeta"], meta["wmms"]
    NPAD = NT * 128
    KIN = 4 if layer == 1 else 128   # contraction width of dense transform

    nc = bacc.Bacc("TRN2", target_bir_lowering=False, debug=False,
                   num_swdge_queues=4, dynamic_dma_scratch_size=32768)
    tab_d = nc.dram_tensor("tab", [N, 128], BF16, kind="ExternalInput")
    idx_d = nc.dram_tensor("idx", [128, TOTCH * 8], I16, kind="ExternalInput")
    dstl_d = nc.dram_tensor("dstl", [128, TOTCH], F32, kind="ExternalInput")
    iota_d = nc.dram_tensor("iota", [128, WT * 128], BF16, kind="ExternalInput")
    w_d = nc.dram_tensor("w", [KIN, 128], F32, kind="ExternalInput")
    if has_bias:
        brow_d = nc.dram_tensor("brow", [1, 128], F32, kind="ExternalInput")
        sqd_d = nc.dram_tensor("sqd", [1, NPAD], F32, kind="ExternalInput")
    ident_d = nc.dram_tensor("ident", [128, 128], F32, kind="ExternalInput")
    identb_d = nc.dram_tensor("identb", [128, 128], BF16, kind="ExternalInput")
    own_d = nc.dram_tensor("own", [NPC, 128], BF16, kind="ExternalInput")
    if layer == 1:
        ivd_d = nc.dram_tensor("ivd", [128, NT], F32, kind="ExternalInput")
        out_d = nc.dram_tensor("h1s", [NPC, 128], BF16, kind="ExternalOutput")
    else:
        poolw_d = nc.dram_tensor("poolw", [NT, 128, cfg.G], F32, kind="ExternalInput")
        wl_d = nc.dram_tensor("wl", [128, cfg.OUT], F32, kind="ExternalInput")
        out_d = nc.dram_tensor("out", [cfg.G, cfg.OUT], F32, kind="ExternalOutput")

    relu = mybir.ActivationFunctionType.Relu

    with tile.TileContext(nc) as tc:
        nc.gpsimd.load_library(mlp)
        with ExitStack() as ctx:
            const = ctx.enter_context(tc.tile_pool(name="const", bufs=1))
            sb = ctx.enter_context(tc.tile_pool(name="sb", bufs=1))
            msgp = ctx.enter_context(tc.tile_pool(name="msg", bufs=2))
            ohp = ctx.enter_context(tc.tile_pool(name="oh", bufs=4))
            asbp = ctx.enter_context(tc.tile_pool(name="asb", bufs=2))
            rlp = ctx.enter_context(tc.tile_pool(name="rl", bufs=2))
            stp = ctx.enter_context(tc.tile_pool(name="st", bufs=2))
            aggp = ctx.enter_context(tc.tile_pool(name="agg", bufs=4, space="PSUM"))
            p2p = ctx.enter_context(tc.tile_pool(name="p2", bufs=1, space="PSUM"))
            trp = ctx.enter_context(tc.tile_pool(name="tr", bufs=2, space="PSUM"))

            # constants / whole-kernel inputs
            idx_t = const.tile([128, TOTCH * 8], I16)
            nc.sync.dma_start(idx_t[:], idx_d[:])
            dstl_t = const.tile([128, TOTCH], F32)
            nc.sync.dma_start(dstl_t[:], dstl_d[:])
            iota_t = const.tile([128, WT * 128], BF16)
            nc.sync.dma_start(iota_t[:], iota_d[:])
            zc_t = const.tile([1, 512], BF16)
            nc.vector.memset(zc_t[:], 0.0)
            identb_t = const.tile([128, 128], BF16)
            nc.sync.dma_start(identb_t[:], identb_d[:])
            ownp = ctx.enter_context(tc.tile_pool(name="own", bufs=3))
            w_t = const.tile([KIN, 128], F32)
            nc.sync.dma_start(w_t[:], w_d[:])
            ident_t = const.tile([128, 128], F32)
            nc.sync.dma_start(ident_t[:], ident_d[:])
            if has_bias:
                brow_t = const.tile([1, 128], F32)
                nc.sync.dma_start(brow_t[:], brow_d[:])
                sqd_t = const.tile([1, NPAD], F32)
                nc.sync.dma_start(sqd_t[:], sqd_d[:])
            if layer == 1:
                ivd_t = const.tile([128, NT], F32)
                nc.sync.dma_start(ivd_t[:], ivd_d[:])
            else:
                wl_t = const.tile([128, cfg.OUT], F32)
                nc.sync.dma_start(wl_t[:], wl_d[:])
                pwp = ctx.enter_context(tc.tile_pool(name="pw", bufs=3))
                plp = ctx.enter_context(tc.tile_pool(name="pl", bufs=1, space="PSUM"))
                pooled_ps = plp.tile([128, cfg.G], F32)

            # ring capacity / packet limits: <=48 chunks (6144 idxs, ~385
            # descriptors per engine ring of 512) per call, one descriptor per
            # packet, spread over the 4 SWDGE queues.
            CALL_CHUNKS = 48
            gq = 0
            for w, wtiles in enumerate(waves):
                msg = msgp.tile([128, CW, 128], BF16, tag="msg")
                for s in range(NSC):
                    icol0, nidx, mcol0, nch = gmeta[w][s]
                    if nidx == 0:
                        continue
                    r0 = s * cfg.SC_SIZE
                    r1 = min(N, r0 + cfg.SC_SIZE)
                    for cb in range(0, nch, CALL_CHUNKS):
                        ce = min(cb + CALL_CHUNKS, nch)
                        ni = (ce - cb) * 128
                        nc.gpsimd.dma_gather(
                            msg[:, mcol0 + cb:mcol0 + ce, :],
                            tab_d[r0:r1, :],
                            idx_t[:, icol0 + cb * 8:icol0 + cb * 8 + ni // 16],
                            ni, ni, 128,
                            single_packet=False,
                            queue_num=gq % 4,
                        )
                        gq += 1
                # each psum bank is zeroed by one full-width PE matmul; all
                # chunk matmuls then accumulate (start=False) in any order,
                # windows spanning several tiles within a bank in one matmul
                mms = wmms[w]
                aggs = [aggp.tile([KIN, 512], F32, tag="agg", name=f"agg_w{w}_{h}")
                        for h in range((len(wtiles) + 3) // 4)]
                for h, agg in enumerate(aggs):
                    nc.tensor.matmul(agg[:], zc_t[0:1, 0:KIN], zc_t[0:1, 0:512],
                                     start=True, stop=False,
                                     skip_group_check=True)
                for mcol, j0, wid, gcol in mms:
                    oh = ohp.tile([128, wid * 128], BF16, tag="oh")
                    nc.vector.tensor_scalar(
                        oh[:], iota_t[:, j0 * 128:(j0 + wid) * 128],
                        dstl_t[:, gcol:gcol + 1], None,
                        mybir.AluOpType.is_equal)
                    agg = aggs[j0 // 4]
                    psl = agg[:, (j0 % 4) * 128:(j0 % 4 + wid) * 128]
                    nc.tensor.matmul(
                        psl, msg[:, mcol, 0:KIN], oh[:],
                        start=False, stop=False, skip_group_check=True)
                # self-loop term: own table rows via identity matmul; the last
                # self-matmul of each psum bank closes its accumulation group
                for j, t in enumerate(wtiles):
                    rows = min(128, NPC - t * 128)
                    own_t = ownp.tile([128, 128], BF16, tag="own")
                    nc.sync.dma_start(own_t[0:rows, :],
                                      own_d[t * 128:t * 128 + rows, :])
                    psl = aggs[j // 4][:, (j % 4) * 128:(j % 4) * 128 + 128]
                    nc.tensor.matmul(
                        psl, own_t[0:rows, 0:KIN], identb_t[0:rows, :],
                        start=False,
                        stop=(j % 4 == 3 or j == len(wtiles) - 1),
                        skip_group_check=True)
                for j, t in enumerate(wtiles):
                    psl = aggs[j // 4][:, (j % 4) * 128:(j % 4) * 128 + 128]
                    agg_sb = asbp.tile([KIN, 128], F32, tag="asb")
                    nc.scalar.activation(agg_sb[:], psl,
                                         mybir.ActivationFunctionType.Copy)
                    p2 = p2p.tile([128, 128], F32, tag="p2")
                    nc.tensor.matmul(p2[:], w_t[:], agg_sb[:],
                                     start=True, stop=not has_bias)
                    if has_bias:
                        nc.tensor.matmul(p2[:], brow_t[:],
                                         sqd_t[0:1, t * 128:t * 128 + 128],
                                         start=False, stop=True)
                    relu_sb = rlp.tile([128, 128], F32, tag="rl")
                    nc.scalar.activation(relu_sb[:], p2[:], relu)
                    tnm = trp.tile([128, 128], F32, tag="tr")
                    nc.tensor.transpose(tnm[:], relu_sb[:], ident_t[:])
                    if layer == 1:
                        if j == 0:
                            stage = stp.tile([128, WT * 128], BF16, tag="stage")
                        nc.scalar.activation(
                            stage[:, j * 128:j * 128 + 128], tnm[:],
                            mybir.ActivationFunctionType.Copy,
                            scale=ivd_t[:, t:t + 1])
                    else:
                        tnm_sb = stp.tile([128, 128], F32, tag="tnm")
                        nc.scalar.activation(tnm_sb[:], tnm[:],
                                             mybir.ActivationFunctionType.Copy)
                        pw_t = pwp.tile([128, cfg.G], F32, tag="pw")
                        nc.sync.dma_start(pw_t[:], poolw_d[t])
                        nc.tensor.matmul(pooled_ps[:], tnm_sb[:], pw_t[:],
                                         start=(t == 0), stop=(t == NT - 1),
                                         skip_group_check=True)
                if layer == 1:
                    # store wave's node-major rows (bf16)
                    base = wtiles[0] * 128
                    nfull = sum(1 for t in wtiles
                                if (t + 1) * 128 <= NPC)
                    if nfull:
                        dst_ap = out_d[base:base + nfull * 128, :].rearrange(
                            "(j p) f -> p j f", p=128)
                        nc.sync.dma_start(dst_ap, stage[:, 0:nfull * 128]
                                          .rearrange("p (j f) -> p j f", f=128))
                    for j, t in enumerate(wtiles):
                        if (t + 1) * 128 <= NPC:
                            continue
                        rows = NPC - t * 128
                        if rows > 0:
                            nc.sync.dma_start(
                                out_d[t * 128:t * 128 + rows, :],
                                stage[0:rows, j * 128:(j + 1) * 128])

            if layer == 2:
                pooled_sb = sb.tile([128, cfg.G], F32)
                nc.any.tensor_copy(pooled_sb[:], pooled_ps[:])
                outp = p2p.tile([128, 128], F32, tag="p2")
                nc.tensor.matmul(outp[0:cfg.G, 0:cfg.OUT], pooled_sb[:], wl_t[:],
                                 start=True, stop=True, skip_group_check=True)
                out_sb = sb.tile([cfg.G, cfg.OUT], F32)
                nc.any.tensor_copy(out_sb[:], outp[0:cfg.G, 0:cfg.OUT])
                nc.sync.dma_start(out_d[:], out_sb[:])

    nc.compile()
    return nc


# --------------------------------------------------------------------------
# driver
# --------------------------------------------------------------------------

def _run(cfg, meta, W1, b1, W2, b2, Wl, bl, runner):
    NC = cfg.NCORES
    has_b1 = bool(np.any(np.asarray(b1)))
    has_b2 = bool(np.any(np.asarray(b2)))

    assert cfg.DIN <= 4
    W1p = np.zeros((4, 128), dtype=np.float32)
    W1p[:cfg.DIN] = np.asarray(W1, dtype=np.float32)

    identb = np.eye(128, dtype=BF16_NP)
    NPC = cfg.NPC
    nc1 = build_layer(cfg, meta, 1, has_b1)
    in_maps = []
    for c in range(NC):
        m = dict(
            tab=meta["x_pad"], idx=meta["idx_wrap"][c],
            dstl=meta["dstl_wrap"][c], iota=meta["iota"],
            w=W1p, ivd=meta["invdeg_col"][c], ident=meta["ident"],
            identb=identb, own=meta["x_pad"][c * NPC:(c + 1) * NPC],
        )
        if has_b1:
            m["brow"] = np.asarray(b1, np.float32).reshape(1, 128)
            m["sqd"] = meta["sqrtdeg_row"][c]
        in_maps.append(m)
    res1 = runner(nc1, in_maps)
    h1s = np.concatenate([res1[c]["h1s"] for c in range(NC)], axis=0)

    nc2 = build_layer(cfg, meta, 2, has_b2)
    in_maps = []
    for c in range(NC):
        m = dict(
            tab=h1s, idx=meta["idx_wrap"][c],
            dstl=meta["dstl_wrap"][c], iota=meta["iota"],
            w=np.asarray(W2, np.float32), ident=meta["ident"],
            identb=identb, own=h1s[c * NPC:(c + 1) * NPC],
            poolw=meta["poolw"][c], wl=np.asarray(Wl, np.float32),
        )
        if has_b2:
            m["brow"] = np.asarray(b2, np.float32).reshape(1, 128)
            m["sqd"] = meta["sqrtdeg_row"][c]
        in_maps.append(m)
    res2 = runner(nc2, in_maps)
    total = np.sum([res2[c]["out"] for c in range(NC)], axis=0)
    return (total + np.asarray(bl, np.float32)[None, :]).astype(np.float32)


def _hw_runner(nc, in_maps):
    res = run_bass_kernel_spmd(nc, in_maps, core_ids=list(range(len(in_maps))))
    return res.results


def kernel(x, edge_index, batch, W1, b1, W2, b2, Wl, bl):
    cfg = FULL
    meta = preprocess(cfg, x, edge_index, batch)
    return _run(cfg, meta, W1, b1, W2, b2, Wl, bl, _hw_runner)

